# revision 7
# baseline (speedup 1.0000x reference)
"""DeepSeek-V3 MoE routing kernel for Trainium2 (Bass/Tile), 8-core SPMD.

Reference semantics (per token, E=256 experts, G=8 groups of 32):
  scores = sigmoid(logits); swb = scores + bias
  group_score[g] = sum of top-2 of swb within group g
  keep top-4 groups; among kept experts take top-8 by swb
  s = scores * onehot(top8); out_vals = sort_desc(s)/(sum(s)+1e-20)*2.5
  out_idx = indices in descending-s order

Sharding: tokens split evenly across 8 NeuronCores (data parallel),
bias replicated.  Inside a core: tiles of 128 tokens (partition dim) x
256 experts (free dim), processed in groups of TB tiles so elementwise
work batches into wide DVE/Pool/ACT instructions.
"""

import numpy as np

T_FULL = 131072
E = 256
G = 8
EG = 32
N_CORES = 8
T_CORE = T_FULL // N_CORES
P = 128
NEG = -1.0e30
TB = 8  # tiles per batch group


def build_bass(n_tokens: int):
    """Build the single-core Bass module processing [n_tokens, 256]."""
    from contextlib import ExitStack

    import concourse.bacc as bacc
    import concourse.mybir as mybir
    import concourse.tile as tile

    f32 = mybir.dt.float32
    A = mybir.AluOpType
    AX = mybir.AxisListType
    AF = mybir.ActivationFunctionType

    assert n_tokens % (P * TB) == 0
    n_groups = n_tokens // (P * TB)
    W = TB * E  # batched free width

    nc = bacc.Bacc("TRN2", target_bir_lowering=False, debug=False)

    logits_d = nc.dram_tensor("logits", [n_tokens, E], f32, kind="ExternalInput").ap()
    bias_d = nc.dram_tensor("bias", [E], f32, kind="ExternalInput").ap()
    idx_d = nc.dram_tensor("idx", [n_tokens, 8], mybir.dt.int32, kind="ExternalOutput").ap()
    vals_d = nc.dram_tensor("vals", [n_tokens, 8], f32, kind="ExternalOutput").ap()

    with tile.TileContext(nc) as tc, ExitStack() as ctx:
        setup = ctx.enter_context(tc.tile_pool(name="setup", bufs=1))
        big = ctx.enter_context(tc.tile_pool(name="big", bufs=2))
        small = ctx.enter_context(tc.tile_pool(name="small", bufs=3))

        # bias broadcast [128, TB*256] (TB copies along free dim)
        bias_row = setup.tile([1, W], f32)
        b2 = bias_d.rearrange("(a b) -> a b", a=1)
        for j in range(TB):
            nc.sync.dma_start(bias_row[:, j * E:(j + 1) * E], b2)
        bias_bc = setup.tile([P, W], f32)
        nc.gpsimd.partition_broadcast(bias_bc[:], bias_row[:], channels=P)

        for i in range(n_groups):
            rows = slice(i * P * TB, (i + 1) * P * TB)
            # DRAM view: [p, j, e] with token = i*P*TB + j*P + p
            dview = logits_d[rows, :].rearrange("(j p) e -> p j e", p=P)

            lg = big.tile([P, W], f32, tag="lg")
            nc.sync.dma_start(lg[:].rearrange("p (j e) -> p j e", j=TB), dview)

            scores = big.tile([P, W], f32, tag="scores")
            nc.scalar.activation(scores[:], lg[:], AF.Sigmoid)

            swb = big.tile([P, W], f32, tag="swb")
            nc.gpsimd.tensor_add(swb[:], scores[:], bias_bc[:])
            swb4 = swb[:].rearrange("p (j g e) -> p j g e", j=TB, g=G)

            m1 = small.tile([P, TB * G], f32, tag="m1")
            nc.vector.tensor_reduce(m1[:].rearrange("p (j g) -> p j g", j=TB),
                                    swb4, axis=AX.X, op=A.max)

            swb2 = big.tile([P, W], f32, tag="swb2")
            for j in range(TB):
                nc.vector.match_replace(
                    out=swb2[:, j * E:(j + 1) * E],
                    in_to_replace=m1[:, j * G:(j + 1) * G],
                    in_values=swb[:, j * E:(j + 1) * E],
                    imm_value=NEG)

            m2 = small.tile([P, TB * G], f32, tag="m2")
            nc.vector.tensor_reduce(
                m2[:].rearrange("p (j g) -> p j g", j=TB),
                swb2[:].rearrange("p (j g e) -> p j g e", j=TB, g=G),
                axis=AX.X, op=A.max)

            gs = small.tile([P, TB * G], f32, tag="gs")
            nc.vector.tensor_add(gs[:], m1[:], m2[:])

            gm8 = small.tile([P, TB * 8], f32, tag="gm8")
            for j in range(TB):
                nc.vector.max(out=gm8[:, j * 8:(j + 1) * 8],
                              in_=gs[:, j * G:(j + 1) * G])

            # cmp = 1.0 where group NOT selected (gs < 4th-largest)
            tg = gm8[:].rearrange("p (j k) -> p j k", j=TB)[:, :, 3]  # [P, TB]
            cmp = small.tile([P, TB * G], f32, tag="cmp")
            nc.vector.tensor_tensor(
                out=cmp[:].rearrange("p (j g) -> p j g", j=TB),
                in0=gs[:].rearrange("p (j g) -> p j g", j=TB),
                in1=tg.to_broadcast([P, TB, G]),
                op=A.is_lt)

            # goff = cmp * NEG; swbm = swb + goff (masked groups -> -1e30)
            goff = small.tile([P, TB * G], f32, tag="goff")
            nc.vector.tensor_scalar(goff[:], cmp[:], NEG, None, op0=A.mult)
            swbm = big.tile([P, W], f32, tag="swbm")
            nc.gpsimd.tensor_add(
                swbm[:].rearrange("p (j g e) -> p j g e", j=TB, g=G),
                swb4,
                goff[:].rearrange("p (j g) -> p j g", j=TB).to_broadcast([P, TB, G, EG]))

            v8b = small.tile([P, TB * 8], f32, tag="v8b")
            for j in range(TB):
                nc.vector.max(out=v8b[:, j * 8:(j + 1) * 8],
                              in_=swbm[:, j * E:(j + 1) * E])

            # negated next-below-t8 threshold: -(t8 - |t8|*1.5*2^-23),
            # so that (swbm >= t8) <=> Sign(swbm + nt8p) == +1 exactly
            # (1-3 ulp below t8, strictly above any value < t8 except
            # pathological 1-ulp near-ties).
            t8c = small.tile([P, TB], f32, tag="t8c")
            nc.vector.tensor_copy(t8c[:], v8b[:, 7::8])
            t8n = small.tile([P, TB], f32, tag="t8n")
            nc.vector.tensor_scalar(t8n[:], t8c[:], -1.0, None, op0=A.mult)
            t8a = small.tile([P, TB], f32, tag="t8a")
            nc.vector.tensor_tensor(t8a[:], t8c[:], t8n[:], op=A.max)
            nt8p = small.tile([P, TB], f32, tag="nt8p")
            nc.vector.scalar_tensor_tensor(
                out=nt8p[:], in0=t8a[:], scalar=float(1.5 * 2.0 ** -23),
                in1=t8c[:], op0=A.mult, op1=A.subtract)

            # sgn = Sign(swbm - t8_prev) in {-1, 0, +1}; +1 exactly at the
            # selected top-8 positions.  s = scores * sgn keeps selected
            # scores positive; everything else is <= 0 and never enters
            # the final top-8 (scores > 0 always).
            sgn = big.tile([P, W], f32, tag="sgn")
            for j in range(TB):
                nc.scalar.activation(
                    sgn[:, j * E:(j + 1) * E], swbm[:, j * E:(j + 1) * E],
                    AF.Sign, bias=nt8p[:, j:j + 1], scale=1.0)

            s = big.tile([P, W], f32, tag="s")
            nc.gpsimd.tensor_tensor(s[:], scores[:], sgn[:], op=A.mult)

            v8u = small.tile([P, TB * 8], f32, tag="v8u")
            for j in range(TB):
                nc.vector.max(out=v8u[:, j * 8:(j + 1) * 8],
                              in_=s[:, j * E:(j + 1) * E])

            idx8 = small.tile([P, TB * 8], mybir.dt.uint32, tag="idx8")
            for j in range(TB):
                nc.vector.max_index(out=idx8[:, j * 8:(j + 1) * 8],
                                    in_max=v8u[:, j * 8:(j + 1) * 8],
                                    in_values=s[:, j * E:(j + 1) * E])

            ssum = small.tile([P, TB], f32, tag="ssum")
            nc.vector.tensor_reduce(ssum[:],
                                    v8u[:].rearrange("p (j k) -> p j k", j=TB),
                                    axis=AX.X, op=A.add)

            rec = small.tile([P, TB], f32, tag="rec")
            nc.vector.reciprocal(rec[:], ssum[:])

            vals8 = small.tile([P, TB * 8], f32, tag="vals8")
            nc.vector.scalar_tensor_tensor(
                out=vals8[:].rearrange("p (j k) -> p j k", j=TB),
                in0=v8u[:].rearrange("p (j k) -> p j k", j=TB),
                scalar=2.5,
                in1=rec[:].to_broadcast([P, TB, 8]),
                op0=A.mult, op1=A.mult)

            oi = idx_d[rows, :].rearrange("(j p) k -> p j k", p=P)
            ov = vals_d[rows, :].rearrange("(j p) k -> p j k", p=P)
            nc.sync.dma_start(
                oi, idx8[:].bitcast(mybir.dt.int32).rearrange("p (j k) -> p j k", j=TB))
            nc.sync.dma_start(ov, vals8[:].rearrange("p (j k) -> p j k", j=TB))

    nc.compile()
    return nc


_NC_CACHE = {}


def _get_nc(n_tokens: int):
    if n_tokens not in _NC_CACHE:
        _NC_CACHE[n_tokens] = build_bass(n_tokens)
    return _NC_CACHE[n_tokens]


def run_spmd(nc, logits, bias, trace=False):
    from concourse import bass_utils

    n = logits.shape[0] // N_CORES
    in_maps = [
        {"logits": np.ascontiguousarray(logits[c * n:(c + 1) * n]),
         "bias": np.ascontiguousarray(bias)}
        for c in range(N_CORES)
    ]
    res = bass_utils.run_bass_kernel_spmd(nc, in_maps, list(range(N_CORES)),
                                          trace=trace)
    idx = np.concatenate([r["idx"] for r in res.results], axis=0)
    vals = np.concatenate([r["vals"] for r in res.results], axis=0)
    return (idx.astype(np.int32), vals.astype(np.float32)), res


def kernel(logits, e_score_correction_bias):
    logits = np.asarray(logits, dtype=np.float32)
    bias = np.asarray(e_score_correction_bias, dtype=np.float32)
    assert logits.shape == (T_FULL, E)
    nc = _get_nc(T_CORE)
    (idx, vals), _ = run_spmd(nc, logits, bias)
    return idx, vals


# revision 8
# speedup vs baseline: 1.0014x; 1.0014x over previous
"""DeepSeek-V3 MoE routing kernel for Trainium2 (Bass/Tile), 8-core SPMD.

Reference semantics (per token, E=256 experts, G=8 groups of 32):
  scores = sigmoid(logits); swb = scores + bias
  group_score[g] = sum of top-2 of swb within group g
  keep top-4 groups; among kept experts take top-8 by swb
  s = scores * onehot(top8); out_vals = sort_desc(s)/(sum(s)+1e-20)*2.5
  out_idx = indices in descending-s order

Sharding: tokens split evenly across 8 NeuronCores (data parallel),
bias replicated.  Inside a core: tiles of 128 tokens (partition dim) x
256 experts (free dim), processed in groups of TB tiles so elementwise
work batches into wide DVE/Pool/ACT instructions.

Engine split: ACT = sigmoid + sign-select, Pool(GpSimd) = broadcast
adds/multiplies, DVE = reductions, match_replace, max8, find_index8.
Big tiles are reused in place (swb -> swbm -> sgn share one tile;
scores -> s share one tile) so three tile groups can be in flight.
"""

import numpy as np

T_FULL = 131072
E = 256
G = 8
EG = 32
N_CORES = 8
T_CORE = T_FULL // N_CORES
P = 128
NEG = -1.0e30
TB = 8  # tiles per batch group
SPLIT = 2  # tiles per Pool sub-op for swbm/s (pipelining granularity)


def build_bass(n_tokens: int):
    """Build the single-core Bass module processing [n_tokens, 256]."""
    from contextlib import ExitStack

    import concourse.bacc as bacc
    import concourse.mybir as mybir
    import concourse.tile as tile

    f32 = mybir.dt.float32
    A = mybir.AluOpType
    AX = mybir.AxisListType
    AF = mybir.ActivationFunctionType

    assert n_tokens % (P * TB) == 0
    n_groups = n_tokens // (P * TB)
    W = TB * E  # batched free width

    nc = bacc.Bacc("TRN2", target_bir_lowering=False, debug=False)

    logits_d = nc.dram_tensor("logits", [n_tokens, E], f32, kind="ExternalInput").ap()
    bias_d = nc.dram_tensor("bias", [E], f32, kind="ExternalInput").ap()
    idx_d = nc.dram_tensor("idx", [n_tokens, 8], mybir.dt.int32, kind="ExternalOutput").ap()
    vals_d = nc.dram_tensor("vals", [n_tokens, 8], f32, kind="ExternalOutput").ap()

    with tile.TileContext(nc) as tc, ExitStack() as ctx:
        setup = ctx.enter_context(tc.tile_pool(name="setup", bufs=1))
        big = ctx.enter_context(tc.tile_pool(name="big", bufs=3))
        small = ctx.enter_context(tc.tile_pool(name="small", bufs=3))

        # bias broadcast [128, TB*256] (TB copies along free dim)
        bias_row = setup.tile([1, W], f32)
        b2 = bias_d.rearrange("(a b) -> a b", a=1)
        for j in range(TB):
            nc.sync.dma_start(bias_row[:, j * E:(j + 1) * E], b2)
        bias_bc = setup.tile([P, W], f32)
        nc.gpsimd.partition_broadcast(bias_bc[:], bias_row[:], channels=P)
        negc = setup.tile([P, 1], f32)
        nc.vector.memset(negc[:], NEG)

        for i in range(n_groups):
            rows = slice(i * P * TB, (i + 1) * P * TB)
            # DRAM view: [p, j, e] with token = i*P*TB + j*P + p
            dview = logits_d[rows, :].rearrange("(j p) e -> p j e", p=P)

            # scores tile: logits in, sigmoid in place, later s = scores*sgn
            scores = big.tile([P, W], f32, tag="scores")
            nc.sync.dma_start(scores[:].rearrange("p (j e) -> p j e", j=TB), dview)
            nc.scalar.activation(scores[:], scores[:], AF.Sigmoid)

            # swb tile: swb = scores + bias; later swbm, then sgn in place
            swb = big.tile([P, W], f32, tag="swb")
            nc.gpsimd.tensor_add(swb[:], scores[:], bias_bc[:])
            swb4 = swb[:].rearrange("p (j g e) -> p j g e", j=TB, g=G)

            m1 = small.tile([P, TB * G], f32, tag="m1")
            nc.vector.tensor_reduce(m1[:].rearrange("p (j g) -> p j g", j=TB),
                                    swb4, axis=AX.X, op=A.max)

            swb2 = big.tile([P, W], f32, tag="swb2")
            for j in range(TB):
                nc.vector.match_replace(
                    out=swb2[:, j * E:(j + 1) * E],
                    in_to_replace=m1[:, j * G:(j + 1) * G],
                    in_values=swb[:, j * E:(j + 1) * E],
                    imm_value=NEG)

            m2 = small.tile([P, TB * G], f32, tag="m2")
            nc.vector.tensor_reduce(
                m2[:].rearrange("p (j g) -> p j g", j=TB),
                swb2[:].rearrange("p (j g e) -> p j g e", j=TB, g=G),
                axis=AX.X, op=A.max)

            gs = small.tile([P, TB * G], f32, tag="gs")
            nc.vector.tensor_add(gs[:], m1[:], m2[:])

            gm8 = small.tile([P, TB * 8], f32, tag="gm8")
            for j in range(TB):
                nc.vector.max(out=gm8[:, j * 8:(j + 1) * 8],
                              in_=gs[:, j * G:(j + 1) * G])

            # cmp = 1.0 where group NOT selected (gs < 4th-largest)
            tg = gm8[:, 3::8]  # [P, TB]
            cmp = small.tile([P, TB * G], f32, tag="cmp")
            nc.vector.tensor_tensor(
                out=cmp[:].rearrange("p (j g) -> p j g", j=TB),
                in0=gs[:].rearrange("p (j g) -> p j g", j=TB),
                in1=tg.to_broadcast([P, TB, G]),
                op=A.is_lt)
            goff = small.tile([P, TB * G], f32, tag="goff")
            nc.gpsimd.tensor_tensor(goff[:], cmp[:],
                                    negc[:].to_broadcast([P, TB * G]), op=A.mult)

            # swbm = swb + goff (in place over swb; masked groups -> -1e30)
            SW = SPLIT * E
            for j in range(0, TB, SPLIT):
                sl = slice(j * E, j * E + SW)
                nc.gpsimd.tensor_add(
                    swb[:, sl].rearrange("p (j g e) -> p j g e", j=SPLIT, g=G),
                    swb[:, sl].rearrange("p (j g e) -> p j g e", j=SPLIT, g=G),
                    goff[:, j * G:(j + SPLIT) * G]
                    .rearrange("p (j g) -> p j g", j=SPLIT)
                    .to_broadcast([P, SPLIT, G, EG]))

            v8b = small.tile([P, TB * 8], f32, tag="v8b")
            for j in range(TB):
                nc.vector.max(out=v8b[:, j * 8:(j + 1) * 8],
                              in_=swb[:, j * E:(j + 1) * E])

            # negated next-below-t8 threshold: -(t8 - |t8|*1.5*2^-23) =
            # max(t8*(c-1), t8*(-c-1)) with c = 1.5*2^-23, computed without
            # abs: nt8p = max(t8*(c-1), -t8*(c+1)).
            c = 1.5 * 2.0 ** -23
            t8lo = small.tile([P, TB], f32, tag="t8lo")
            nc.vector.tensor_scalar(t8lo[:], v8b[:, 7::8], c - 1.0, None, op0=A.mult)
            t8hi = small.tile([P, TB], f32, tag="t8hi")
            nc.vector.tensor_scalar(t8hi[:], v8b[:, 7::8], -c - 1.0, None, op0=A.mult)
            nt8p = small.tile([P, TB], f32, tag="nt8p")
            nc.vector.tensor_tensor(nt8p[:], t8lo[:], t8hi[:], op=A.max)

            # sgn = Sign(swbm + nt8p) in {-1,0,+1}, +1 exactly at selected
            # top-8 positions (in place over swbm).
            for j in range(TB):
                nc.scalar.activation(
                    swb[:, j * E:(j + 1) * E], swb[:, j * E:(j + 1) * E],
                    AF.Sign, bias=nt8p[:, j:j + 1], scale=1.0)

            # s = scores * sgn (in place over scores): selected scores stay
            # positive, all else <= 0 and never enters the final top-8.
            for j in range(0, TB, SPLIT):
                sl = slice(j * E, j * E + SW)
                nc.gpsimd.tensor_tensor(scores[:, sl], scores[:, sl],
                                        swb[:, sl], op=A.mult)

            v8u = small.tile([P, TB * 8], f32, tag="v8u")
            for j in range(TB):
                nc.vector.max(out=v8u[:, j * 8:(j + 1) * 8],
                              in_=scores[:, j * E:(j + 1) * E])

            idx8 = small.tile([P, TB * 8], mybir.dt.uint32, tag="idx8")
            for j in range(TB):
                nc.vector.max_index(out=idx8[:, j * 8:(j + 1) * 8],
                                    in_max=v8u[:, j * 8:(j + 1) * 8],
                                    in_values=scores[:, j * E:(j + 1) * E])

            # vals = v8u * (2.5 / ssum):  rec25 = 1/(ssum*0.4)
            ssum = small.tile([P, TB], f32, tag="ssum")
            nc.vector.tensor_reduce(ssum[:],
                                    v8u[:].rearrange("p (j k) -> p j k", j=TB),
                                    axis=AX.X, op=A.add)
            ssum4 = small.tile([P, TB], f32, tag="ssum4")
            nc.vector.tensor_scalar(ssum4[:], ssum[:], 0.4, None, op0=A.mult)
            rec = small.tile([P, TB], f32, tag="rec")
            nc.vector.reciprocal(rec[:], ssum4[:])

            vals8 = small.tile([P, TB * 8], f32, tag="vals8")
            nc.gpsimd.tensor_tensor(
                vals8[:].rearrange("p (j k) -> p j k", j=TB),
                v8u[:].rearrange("p (j k) -> p j k", j=TB),
                rec[:].to_broadcast([P, TB, 8]), op=A.mult)

            oi = idx_d[rows, :].rearrange("(j p) k -> p j k", p=P)
            ov = vals_d[rows, :].rearrange("(j p) k -> p j k", p=P)
            nc.sync.dma_start(
                oi, idx8[:].bitcast(mybir.dt.int32).rearrange("p (j k) -> p j k", j=TB))
            nc.sync.dma_start(ov, vals8[:].rearrange("p (j k) -> p j k", j=TB))

    nc.compile()
    return nc


_NC_CACHE = {}


def _get_nc(n_tokens: int):
    if n_tokens not in _NC_CACHE:
        _NC_CACHE[n_tokens] = build_bass(n_tokens)
    return _NC_CACHE[n_tokens]


def run_spmd(nc, logits, bias, trace=False):
    from concourse import bass_utils

    n = logits.shape[0] // N_CORES
    in_maps = [
        {"logits": np.ascontiguousarray(logits[c * n:(c + 1) * n]),
         "bias": np.ascontiguousarray(bias)}
        for c in range(N_CORES)
    ]
    res = bass_utils.run_bass_kernel_spmd(nc, in_maps, list(range(N_CORES)),
                                          trace=trace)
    idx = np.concatenate([r["idx"] for r in res.results], axis=0)
    vals = np.concatenate([r["vals"] for r in res.results], axis=0)
    return (idx.astype(np.int32), vals.astype(np.float32)), res


def kernel(logits, e_score_correction_bias):
    logits = np.asarray(logits, dtype=np.float32)
    bias = np.asarray(e_score_correction_bias, dtype=np.float32)
    assert logits.shape == (T_FULL, E)
    nc = _get_nc(T_CORE)
    (idx, vals), _ = run_spmd(nc, logits, bias)
    return idx, vals


# revision 10
# speedup vs baseline: 1.0685x; 1.0670x over previous
"""DeepSeek-V3 MoE routing kernel for Trainium2 (Bass/Tile), 8-core SPMD.

Reference semantics (per token, E=256 experts, G=8 groups of 32):
  scores = sigmoid(logits); swb = scores + bias
  group_score[g] = sum of top-2 of swb within group g
  keep top-4 groups; among kept experts take top-8 by swb
  s = scores * onehot(top8); out_vals = sort_desc(s)/(sum(s)+1e-20)*2.5
  out_idx = indices in descending-s order

Sharding: tokens split evenly across 8 NeuronCores (data parallel),
bias replicated.  Inside a core: tiles of 128 tokens (partition dim) x
256 experts (free dim), processed in groups of TB tiles so elementwise
work batches into wide DVE/Pool/ACT instructions.

Engine split: ACT = sigmoid + sign-select, Pool(GpSimd) = broadcast
adds/multiplies, DVE = reductions, match_replace, max8, find_index8.
Big tiles are reused in place (swb -> swbm -> sgn share one tile;
scores -> s share one tile) so three tile groups can be in flight.
"""

import numpy as np

T_FULL = 131072
E = 256
G = 8
EG = 32
N_CORES = 8
T_CORE = T_FULL // N_CORES
P = 128
NEG = -1.0e30
TB = 8  # tiles per batch group
SPLIT = 2  # tiles per Pool sub-op for swbm/s (pipelining granularity)


def build_bass(n_tokens: int):
    """Build the single-core Bass module processing [n_tokens, 256]."""
    from contextlib import ExitStack

    import concourse.bacc as bacc
    import concourse.mybir as mybir
    import concourse.tile as tile

    f32 = mybir.dt.float32
    A = mybir.AluOpType
    AX = mybir.AxisListType
    AF = mybir.ActivationFunctionType

    assert n_tokens % (P * TB) == 0
    n_groups = n_tokens // (P * TB)
    W = TB * E  # batched free width

    nc = bacc.Bacc("TRN2", target_bir_lowering=False, debug=False)

    logits_d = nc.dram_tensor("logits", [n_tokens, E], f32, kind="ExternalInput").ap()
    biasb_d = nc.dram_tensor("biasb", [P, W], f32, kind="ExternalInput").ap()
    idx_d = nc.dram_tensor("idx", [n_tokens, 8], mybir.dt.int32, kind="ExternalOutput").ap()
    vals_d = nc.dram_tensor("vals", [n_tokens, 8], f32, kind="ExternalOutput").ap()

    with tile.TileContext(nc) as tc, ExitStack() as ctx:
        setup = ctx.enter_context(tc.tile_pool(name="setup", bufs=1))
        big = ctx.enter_context(tc.tile_pool(name="big", bufs=4))
        small = ctx.enter_context(tc.tile_pool(name="small", bufs=3))

        # bias pre-broadcast on host: [128, TB*256]
        bias_bc = setup.tile([P, W], f32)
        nc.sync.dma_start(bias_bc[:], biasb_d)
        negc = setup.tile([P, 1], f32)
        nc.vector.memset(negc[:], NEG)

        for i in range(n_groups):
            rows = slice(i * P * TB, (i + 1) * P * TB)
            # DRAM view: [p, j, e] with token = i*P*TB + j*P + p
            dview = logits_d[rows, :].rearrange("(j p) e -> p j e", p=P)

            # scores tile: logits in, sigmoid in place, later s = scores*sgn
            scores = big.tile([P, W], f32, tag="scores")
            nc.sync.dma_start(scores[:].rearrange("p (j e) -> p j e", j=TB), dview)
            nc.scalar.activation(scores[:], scores[:], AF.Sigmoid)

            # swb tile: swb = scores + bias; later swbm, then sgn in place
            swb = big.tile([P, W], f32, tag="swb")
            nc.gpsimd.tensor_add(swb[:], scores[:], bias_bc[:])
            swb4 = swb[:].rearrange("p (j g e) -> p j g e", j=TB, g=G)

            m1 = small.tile([P, TB * G], f32, tag="m1")
            nc.vector.tensor_reduce(m1[:].rearrange("p (j g) -> p j g", j=TB),
                                    swb4, axis=AX.X, op=A.max)

            swb2 = big.tile([P, W], f32, tag="swb2")
            for j in range(TB):
                nc.vector.match_replace(
                    out=swb2[:, j * E:(j + 1) * E],
                    in_to_replace=m1[:, j * G:(j + 1) * G],
                    in_values=swb[:, j * E:(j + 1) * E],
                    imm_value=NEG)

            m2 = small.tile([P, TB * G], f32, tag="m2")
            nc.vector.tensor_reduce(
                m2[:].rearrange("p (j g) -> p j g", j=TB),
                swb2[:].rearrange("p (j g e) -> p j g e", j=TB, g=G),
                axis=AX.X, op=A.max)

            gs = small.tile([P, TB * G], f32, tag="gs")
            nc.vector.tensor_add(gs[:], m1[:], m2[:])

            gm8 = small.tile([P, TB * 8], f32, tag="gm8")
            for j in range(TB):
                nc.vector.max(out=gm8[:, j * 8:(j + 1) * 8],
                              in_=gs[:, j * G:(j + 1) * G])

            # cmp = 1.0 where group NOT selected (gs < 4th-largest)
            tg = gm8[:, 3::8]  # [P, TB]
            cmp = small.tile([P, TB * G], f32, tag="cmp")
            nc.vector.tensor_tensor(
                out=cmp[:].rearrange("p (j g) -> p j g", j=TB),
                in0=gs[:].rearrange("p (j g) -> p j g", j=TB),
                in1=tg.to_broadcast([P, TB, G]),
                op=A.is_lt)
            goff = small.tile([P, TB * G], f32, tag="goff")
            nc.gpsimd.tensor_tensor(goff[:], cmp[:],
                                    negc[:].to_broadcast([P, TB * G]), op=A.mult)

            # swbm = swb + goff (in place over swb; masked groups -> -1e30)
            SW = SPLIT * E
            for j in range(0, TB, SPLIT):
                sl = slice(j * E, j * E + SW)
                nc.gpsimd.tensor_add(
                    swb[:, sl].rearrange("p (j g e) -> p j g e", j=SPLIT, g=G),
                    swb[:, sl].rearrange("p (j g e) -> p j g e", j=SPLIT, g=G),
                    goff[:, j * G:(j + SPLIT) * G]
                    .rearrange("p (j g) -> p j g", j=SPLIT)
                    .to_broadcast([P, SPLIT, G, EG]))

            v8b = small.tile([P, TB * 8], f32, tag="v8b")
            for j in range(TB):
                nc.vector.max(out=v8b[:, j * 8:(j + 1) * 8],
                              in_=swb[:, j * E:(j + 1) * E])

            # negated next-below-t8 threshold: -(t8 - |t8|*1.5*2^-23) =
            # max(t8*(c-1), t8*(-c-1)) with c = 1.5*2^-23, computed without
            # abs: nt8p = max(t8*(c-1), -t8*(c+1)).
            c = 1.5 * 2.0 ** -23
            t8lo = small.tile([P, TB], f32, tag="t8lo")
            nc.vector.tensor_scalar(t8lo[:], v8b[:, 7::8], c - 1.0, None, op0=A.mult)
            t8hi = small.tile([P, TB], f32, tag="t8hi")
            nc.vector.tensor_scalar(t8hi[:], v8b[:, 7::8], -c - 1.0, None, op0=A.mult)
            nt8p = small.tile([P, TB], f32, tag="nt8p")
            nc.vector.tensor_tensor(nt8p[:], t8lo[:], t8hi[:], op=A.max)

            # sgn = Sign(swbm + nt8p) in {-1,0,+1}, +1 exactly at selected
            # top-8 positions (in place over swbm).
            for j in range(TB):
                nc.scalar.activation(
                    swb[:, j * E:(j + 1) * E], swb[:, j * E:(j + 1) * E],
                    AF.Sign, bias=nt8p[:, j:j + 1], scale=1.0)

            # s = scores * sgn (in place over scores): selected scores stay
            # positive, all else <= 0 and never enters the final top-8.
            for j in range(0, TB, SPLIT):
                sl = slice(j * E, j * E + SW)
                nc.gpsimd.tensor_tensor(scores[:, sl], scores[:, sl],
                                        swb[:, sl], op=A.mult)

            v8u = small.tile([P, TB * 8], f32, tag="v8u")
            for j in range(TB):
                nc.vector.max(out=v8u[:, j * 8:(j + 1) * 8],
                              in_=scores[:, j * E:(j + 1) * E])

            idx8 = small.tile([P, TB * 8], mybir.dt.uint32, tag="idx8")
            for j in range(TB):
                nc.vector.max_index(out=idx8[:, j * 8:(j + 1) * 8],
                                    in_max=v8u[:, j * 8:(j + 1) * 8],
                                    in_values=scores[:, j * E:(j + 1) * E])

            # vals = v8u * (2.5 / ssum):  rec25 = 1/(ssum*0.4)
            ssum = small.tile([P, TB], f32, tag="ssum")
            nc.vector.tensor_reduce(ssum[:],
                                    v8u[:].rearrange("p (j k) -> p j k", j=TB),
                                    axis=AX.X, op=A.add)
            ssum4 = small.tile([P, TB], f32, tag="ssum4")
            nc.vector.tensor_scalar(ssum4[:], ssum[:], 0.4, None, op0=A.mult)
            rec = small.tile([P, TB], f32, tag="rec")
            nc.vector.reciprocal(rec[:], ssum4[:])

            vals8 = small.tile([P, TB * 8], f32, tag="vals8")
            nc.gpsimd.tensor_tensor(
                vals8[:].rearrange("p (j k) -> p j k", j=TB),
                v8u[:].rearrange("p (j k) -> p j k", j=TB),
                rec[:].to_broadcast([P, TB, 8]), op=A.mult)

            oi = idx_d[rows, :].rearrange("(j p) k -> p j k", p=P)
            ov = vals_d[rows, :].rearrange("(j p) k -> p j k", p=P)
            nc.sync.dma_start(
                oi, idx8[:].bitcast(mybir.dt.int32).rearrange("p (j k) -> p j k", j=TB))
            nc.sync.dma_start(ov, vals8[:].rearrange("p (j k) -> p j k", j=TB))

    nc.compile()
    return nc


_NC_CACHE = {}


def _get_nc(n_tokens: int):
    if n_tokens not in _NC_CACHE:
        _NC_CACHE[n_tokens] = build_bass(n_tokens)
    return _NC_CACHE[n_tokens]


def run_spmd(nc, logits, bias, trace=False):
    from concourse import bass_utils

    n = logits.shape[0] // N_CORES
    biasb = np.ascontiguousarray(
        np.broadcast_to(np.tile(bias, TB)[None, :], (P, TB * E)).astype(np.float32))
    in_maps = [
        {"logits": np.ascontiguousarray(logits[c * n:(c + 1) * n]),
         "biasb": biasb}
        for c in range(N_CORES)
    ]
    res = bass_utils.run_bass_kernel_spmd(nc, in_maps, list(range(N_CORES)),
                                          trace=trace)
    idx = np.concatenate([r["idx"] for r in res.results], axis=0)
    vals = np.concatenate([r["vals"] for r in res.results], axis=0)
    return (idx.astype(np.int32), vals.astype(np.float32)), res


def kernel(logits, e_score_correction_bias):
    logits = np.asarray(logits, dtype=np.float32)
    bias = np.asarray(e_score_correction_bias, dtype=np.float32)
    assert logits.shape == (T_FULL, E)
    nc = _get_nc(T_CORE)
    (idx, vals), _ = run_spmd(nc, logits, bias)
    return idx, vals


# revision 13
# speedup vs baseline: 1.0976x; 1.0272x over previous
"""DeepSeek-V3 MoE routing kernel for Trainium2 (Bass/Tile), 8-core SPMD.

Reference semantics (per token, E=256 experts, G=8 groups of 32):
  scores = sigmoid(logits); swb = scores + bias
  group_score[g] = sum of top-2 of swb within group g
  keep top-4 groups; among kept experts take top-8 by swb
  s = scores * onehot(top8); out_vals = sort_desc(s)/(sum(s)+1e-20)*2.5
  out_idx = indices in descending-s order

Sharding: tokens split evenly across 8 NeuronCores (data parallel),
bias replicated.  Inside a core: tiles of 128 tokens (partition dim) x
256 experts (free dim), processed in groups of TB tiles so elementwise
work batches into wide DVE/Pool/ACT instructions.

Engine split: ACT = sigmoid + sign-select, Pool(GpSimd) = broadcast
adds/multiplies, DVE = reductions, match_replace, max8, find_index8.
Big tiles are reused in place (swb -> swbm -> sgn share one tile;
scores -> s share one tile) so three tile groups can be in flight.
"""

import numpy as np

T_FULL = 131072
E = 256
G = 8
EG = 32
N_CORES = 8
T_CORE = T_FULL // N_CORES
P = 128
NEG = -1.0e30
TB = 8  # tiles per batch group
SPLIT = 2  # tiles per Pool sub-op for swbm/s (pipelining granularity)


def build_bass(n_tokens: int):
    """Build the single-core Bass module processing [n_tokens, 256]."""
    from contextlib import ExitStack

    import concourse.bacc as bacc
    import concourse.mybir as mybir
    import concourse.tile as tile

    f32 = mybir.dt.float32
    A = mybir.AluOpType
    AX = mybir.AxisListType
    AF = mybir.ActivationFunctionType

    assert n_tokens % (P * TB) == 0
    n_groups = n_tokens // (P * TB)
    W = TB * E  # batched free width

    nc = bacc.Bacc("TRN2", target_bir_lowering=False, debug=False)

    logits_d = nc.dram_tensor("logits", [n_tokens, E], f32, kind="ExternalInput").ap()
    biasb_d = nc.dram_tensor("biasb", [P, W], f32, kind="ExternalInput").ap()
    idx_d = nc.dram_tensor("idx", [n_tokens, 8], mybir.dt.int32, kind="ExternalOutput").ap()
    vals_d = nc.dram_tensor("vals", [n_tokens, 8], f32, kind="ExternalOutput").ap()

    with tile.TileContext(nc) as tc, ExitStack() as ctx:
        setup = ctx.enter_context(tc.tile_pool(name="setup", bufs=1))
        big = ctx.enter_context(tc.tile_pool(name="big", bufs=4))
        small = ctx.enter_context(tc.tile_pool(name="small", bufs=3))

        # bias pre-broadcast on host: [128, TB*256]
        bias_bc = setup.tile([P, W], f32)
        nc.sync.dma_start(bias_bc[:], biasb_d)
        negc = setup.tile([P, 1], f32)
        nc.vector.memset(negc[:], NEG)

        def phase_a(i):
            """Group front half: load .. sign/select launch.

            Returns (scores, v8u-deps...) for phase_b.  Ends with the
            Pool multiply producing s in place over `scores`; DVE work of
            the NEXT group overlaps that latency (software pipelining).
            """
            rows = slice(i * P * TB, (i + 1) * P * TB)
            # DRAM view: [p, j, e] with token = i*P*TB + j*P + p
            dview = logits_d[rows, :].rearrange("(j p) e -> p j e", p=P)

            # scores tile: logits in, sigmoid in place, later s = scores*sgn
            scores = big.tile([P, W], f32, tag="scores")
            nc.sync.dma_start(scores[:].rearrange("p (j e) -> p j e", j=TB), dview)
            nc.scalar.activation(scores[:], scores[:], AF.Sigmoid)

            # swb tile: swb = scores + bias; later swbm, then sgn in place
            swb = big.tile([P, W], f32, tag="swb")
            nc.gpsimd.tensor_add(swb[:], scores[:], bias_bc[:])
            swb4 = swb[:].rearrange("p (j g e) -> p j g e", j=TB, g=G)

            m1 = small.tile([P, TB * G], f32, tag="m1")
            nc.vector.tensor_reduce(m1[:].rearrange("p (j g) -> p j g", j=TB),
                                    swb4, axis=AX.X, op=A.max)

            swb2 = big.tile([P, W], f32, tag="swb2")
            for j in range(TB):
                nc.vector.match_replace(
                    out=swb2[:, j * E:(j + 1) * E],
                    in_to_replace=m1[:, j * G:(j + 1) * G],
                    in_values=swb[:, j * E:(j + 1) * E],
                    imm_value=NEG)

            m2 = small.tile([P, TB * G], f32, tag="m2")
            nc.vector.tensor_reduce(
                m2[:].rearrange("p (j g) -> p j g", j=TB),
                swb2[:].rearrange("p (j g e) -> p j g e", j=TB, g=G),
                axis=AX.X, op=A.max)

            gs = small.tile([P, TB * G], f32, tag="gs")
            nc.vector.tensor_add(gs[:], m1[:], m2[:])

            gm8 = small.tile([P, TB * 8], f32, tag="gm8")
            for j in range(TB):
                nc.vector.max(out=gm8[:, j * 8:(j + 1) * 8],
                              in_=gs[:, j * G:(j + 1) * G])

            # cmp = 1.0 where group NOT selected (gs < 4th-largest)
            tg = gm8[:, 3::8]  # [P, TB]
            cmp = small.tile([P, TB * G], f32, tag="cmp")
            nc.vector.tensor_tensor(
                out=cmp[:].rearrange("p (j g) -> p j g", j=TB),
                in0=gs[:].rearrange("p (j g) -> p j g", j=TB),
                in1=tg.to_broadcast([P, TB, G]),
                op=A.is_lt)
            goff = small.tile([P, TB * G], f32, tag="goff")
            nc.gpsimd.tensor_tensor(goff[:], cmp[:],
                                    negc[:].to_broadcast([P, TB * G]), op=A.mult)

            # swbm = swb + goff (in place over swb; masked groups -> -1e30)
            SW = SPLIT * E
            for j in range(0, TB, SPLIT):
                sl = slice(j * E, j * E + SW)
                nc.gpsimd.tensor_add(
                    swb[:, sl].rearrange("p (j g e) -> p j g e", j=SPLIT, g=G),
                    swb[:, sl].rearrange("p (j g e) -> p j g e", j=SPLIT, g=G),
                    goff[:, j * G:(j + SPLIT) * G]
                    .rearrange("p (j g) -> p j g", j=SPLIT)
                    .to_broadcast([P, SPLIT, G, EG]))

            v8b = small.tile([P, TB * 8], f32, tag="v8b")
            for j in range(TB):
                nc.vector.max(out=v8b[:, j * 8:(j + 1) * 8],
                              in_=swb[:, j * E:(j + 1) * E])

            # negated next-below-t8 threshold: -(t8 - |t8|*1.5*2^-23) =
            # max(t8*(c-1), t8*(-c-1)) with c = 1.5*2^-23, computed without
            # abs: nt8p = max(t8*(c-1), -t8*(c+1)).
            c = 1.5 * 2.0 ** -23
            t8lo = small.tile([P, TB], f32, tag="t8lo")
            nc.vector.tensor_scalar(t8lo[:], v8b[:, 7::8], c - 1.0, None, op0=A.mult)
            t8hi = small.tile([P, TB], f32, tag="t8hi")
            nc.vector.tensor_scalar(t8hi[:], v8b[:, 7::8], -c - 1.0, None, op0=A.mult)
            nt8p = small.tile([P, TB], f32, tag="nt8p")
            nc.vector.tensor_tensor(nt8p[:], t8lo[:], t8hi[:], op=A.max)

            # sgn = Sign(swbm + nt8p) in {-1,0,+1}, +1 exactly at selected
            # top-8 positions (in place over swbm).
            for j in range(TB):
                nc.scalar.activation(
                    swb[:, j * E:(j + 1) * E], swb[:, j * E:(j + 1) * E],
                    AF.Sign, bias=nt8p[:, j:j + 1], scale=1.0)

            # s = scores * sgn (in place over scores): selected scores stay
            # positive, all else <= 0 and never enters the final top-8.
            for j in range(0, TB, SPLIT):
                sl = slice(j * E, j * E + SW)
                nc.gpsimd.tensor_tensor(scores[:, sl], scores[:, sl],
                                        swb[:, sl], op=A.mult)
            return scores

        def phase_b(i, scores):
            """Group back half: final top-8 over s, indices, normalize."""
            rows = slice(i * P * TB, (i + 1) * P * TB)
            v8u = small.tile([P, TB * 8], f32, tag="v8u")
            for j in range(TB):
                nc.vector.max(out=v8u[:, j * 8:(j + 1) * 8],
                              in_=scores[:, j * E:(j + 1) * E])

            idx8 = small.tile([P, TB * 8], mybir.dt.uint32, tag="idx8")
            for j in range(TB):
                nc.vector.max_index(out=idx8[:, j * 8:(j + 1) * 8],
                                    in_max=v8u[:, j * 8:(j + 1) * 8],
                                    in_values=scores[:, j * E:(j + 1) * E])

            # vals = v8u * (2.5 / ssum):  rec25 = 1/(ssum*0.4)
            ssum = small.tile([P, TB], f32, tag="ssum")
            nc.vector.tensor_reduce(ssum[:],
                                    v8u[:].rearrange("p (j k) -> p j k", j=TB),
                                    axis=AX.X, op=A.add)
            ssum4 = small.tile([P, TB], f32, tag="ssum4")
            nc.vector.tensor_scalar(ssum4[:], ssum[:], 0.4, None, op0=A.mult)
            rec = small.tile([P, TB], f32, tag="rec")
            nc.vector.reciprocal(rec[:], ssum4[:])

            vals8 = small.tile([P, TB * 8], f32, tag="vals8")
            nc.gpsimd.tensor_tensor(
                vals8[:].rearrange("p (j k) -> p j k", j=TB),
                v8u[:].rearrange("p (j k) -> p j k", j=TB),
                rec[:].to_broadcast([P, TB, 8]), op=A.mult)

            oi = idx_d[rows, :].rearrange("(j p) k -> p j k", p=P)
            ov = vals_d[rows, :].rearrange("(j p) k -> p j k", p=P)
            nc.sync.dma_start(
                oi, idx8[:].bitcast(mybir.dt.int32).rearrange("p (j k) -> p j k", j=TB))
            nc.sync.dma_start(ov, vals8[:].rearrange("p (j k) -> p j k", j=TB))

        # one-group software pipeline skew: while Pool/ACT produce group
        # i's s tile, DVE runs group i+1's front half.
        prev = None
        for i in range(n_groups):
            sc = phase_a(i)
            if prev is not None:
                phase_b(i - 1, prev)
            prev = sc
        phase_b(n_groups - 1, prev)

    nc.compile()
    return nc


_NC_CACHE = {}


def _get_nc(n_tokens: int):
    if n_tokens not in _NC_CACHE:
        _NC_CACHE[n_tokens] = build_bass(n_tokens)
    return _NC_CACHE[n_tokens]


def run_spmd(nc, logits, bias, trace=False):
    from concourse import bass_utils

    n = logits.shape[0] // N_CORES
    biasb = np.ascontiguousarray(
        np.broadcast_to(np.tile(bias, TB)[None, :], (P, TB * E)).astype(np.float32))
    in_maps = [
        {"logits": np.ascontiguousarray(logits[c * n:(c + 1) * n]),
         "biasb": biasb}
        for c in range(N_CORES)
    ]
    res = bass_utils.run_bass_kernel_spmd(nc, in_maps, list(range(N_CORES)),
                                          trace=trace)
    idx = np.concatenate([r["idx"] for r in res.results], axis=0)
    vals = np.concatenate([r["vals"] for r in res.results], axis=0)
    return (idx.astype(np.int32), vals.astype(np.float32)), res


def kernel(logits, e_score_correction_bias):
    logits = np.asarray(logits, dtype=np.float32)
    bias = np.asarray(e_score_correction_bias, dtype=np.float32)
    assert logits.shape == (T_FULL, E)
    nc = _get_nc(T_CORE)
    (idx, vals), _ = run_spmd(nc, logits, bias)
    return idx, vals
